# revision 41
# baseline (speedup 1.0000x reference)
"""Trainium2 Bass kernel for the MHSA bottleneck block.

Contract: kernel(**inputs) takes the FULL unsharded inputs (as produced by
setup_inputs()) and returns the FULL [64, 2048, 14, 14] float32 output.
Internally shards data-parallel over batch: 8 images per NeuronCore, 8 cores.

Precision plan (error budget 2e-2, lands ~3e-3):
  - conv1 / attention matmuls: bf16 operands, fp32 PSUM accumulate.
  - conv3: fp8e4 DoubleRow (2 rows/cycle), h2 + w3 in fp8. conv3's output
    is small relative to the residual x, so fp8 noise there is damped.
Attention-score algebra: cc = q^T k = h1^T (Wq^T Wk) h1, so only one
projection g = (Wq^T Wk)^T h1 is computed on-chip; cp = pos^T q folds to
P2^T h1 with P2 = Wq^T pos precomputed on host. This removes the q/k
projections entirely.
DMA issue order is arranged so the first conv1 matmul only waits on the
first quarter of x(pair0) + w1t.
"""
import sys

sys.path.insert(0, '/opt/trn_rl_repo')

import numpy as np
import ml_dtypes

BF16 = ml_dtypes.bfloat16
F8E4 = ml_dtypes.float8_e4m3

# Problem constants (hardcoded per the harness contract).
B, CIN, P, H, W = 64, 2048, 512, 14, 14
EPS = 1e-5
N = H * W            # 196 pixels
NCORES = 8
BPC = B // NCORES    # 8 images per core
NPAIR = BPC // 2     # 4 image pairs per core
KC1 = CIN // 128     # 16 input-channel chunks for conv1 / output chunks conv3
PC = P // 128        # 4 chunks of the 512-dim
N2 = 2 * N           # 392 = free dim for image-pair matmuls
N2P = 400            # N2 padded so fp8 DoubleRow strides are 16B-aligned

# n/m chunking of the 196-pixel dim: 128 + 68
NCHUNKS = [(0, 128), (128, 68)]

_CACHE = {}


def _build():
    import concourse.bass as bass  # noqa: F401
    import concourse.mybir as mybir
    import concourse.tile as tile
    from concourse import bacc
    from concourse.masks import make_identity

    f32 = mybir.dt.float32
    bf16 = mybir.dt.bfloat16
    f8 = mybir.dt.float8e4
    DR = mybir.MatmulPerfMode.DoubleRow

    nc = bacc.Bacc(None, target_bir_lowering=False, debug=False)

    # DRAM parameters, partition-major so each DMA is one long line per
    # partition.
    x_d = nc.declare_dram_parameter("x", [128, NPAIR, KC1 * N2], bf16,
                                    isOutput=False)
    w1t_d = nc.declare_dram_parameter("w1t", [128, KC1, P], bf16,
                                      isOutput=False)
    mt_d = nc.declare_dram_parameter("mt", [128, 2 * 2 * P], f8,
                                     isOutput=False)
    wvt_d = nc.declare_dram_parameter("wvt", [128, 2 * 2 * P], f8,
                                      isOutput=False)
    w3t_d = nc.declare_dram_parameter("w3t", [128, 2 * 2 * CIN], f8,
                                      isOutput=False)
    pos_d = nc.declare_dram_parameter("pos", [128, PC, N], bf16,
                                      isOutput=False)
    t1_d = nc.declare_dram_parameter("t1", [128, PC], f32, isOutput=False)
    s2_d = nc.declare_dram_parameter("s2", [128, PC], f32, isOutput=False)
    t2_d = nc.declare_dram_parameter("t2", [128, PC], f32, isOutput=False)
    t3_d = nc.declare_dram_parameter("t3", [128, KC1], f32, isOutput=False)
    y_d = nc.declare_dram_parameter("y", [128, NPAIR, KC1 * N2], bf16,
                                    isOutput=True)

    XCH = 8              # x/w1 startup chunks for pair 0
    KCC = KC1 // XCH     # 2 kc per chunk

    with tile.TileContext(nc) as tc:
        with (
            tc.tile_pool(name="const", bufs=1) as const,
            tc.tile_pool(name="xp", bufs=3) as xp,
            tc.tile_pool(name="x0p", bufs=1) as x0p,
            tc.tile_pool(name="h1p", bufs=2) as h1p,
            tc.tile_pool(name="qkp", bufs=2) as qkp,
            tc.tile_pool(name="h2p", bufs=2) as h2p,
            tc.tile_pool(name="attp", bufs=2) as attp,
            tc.tile_pool(name="outp", bufs=2) as outp,
            tc.tile_pool(name="ps_mm", bufs=4, space="PSUM") as ps_mm,
            tc.tile_pool(name="ps_sm", bufs=3, space="PSUM") as ps_sm,
            tc.tile_pool(name="ps_tr", bufs=1, space="PSUM") as ps_tr,
        ):
            # ---- startup: interleave x(pair0) and w1t chunks so the first
            # conv1 matmul waits on only a small slice of each; the very
            # first chunk is a single kc for the earliest possible start ----
            x0c = []
            w1c = []
            for c in range(XCH):
                xt = x0p.tile([128, KCC, N2], bf16, name=f"x0_{c}")
                wt = const.tile([128, KCC, P], bf16, name=f"w1_{c}")
                for h in range(KCC) if c == 0 else [None]:
                    if h is None:
                        nc.sync.dma_start(
                            out=xt,
                            in_=x_d[:, 0, c * KCC * N2:
                                    (c + 1) * KCC * N2].rearrange(
                                "p (k n) -> p k n", k=KCC))
                        nc.sync.dma_start(
                            out=wt, in_=w1t_d[:, c * KCC:(c + 1) * KCC, :])
                    else:
                        nc.sync.dma_start(
                            out=xt[:, h:h + 1, :],
                            in_=x_d[:, 0, (c * KCC + h) * N2:
                                    (c * KCC + h + 1) * N2].rearrange(
                                "p (k n) -> p k n", k=1))
                        nc.sync.dma_start(
                            out=wt[:, h:h + 1, :],
                            in_=w1t_d[:, c * KCC + h:c * KCC + h + 1, :])
                x0c.append(xt)
                w1c.append(wt)
            t1 = const.tile([128, PC], f32)
            nc.sync.dma_start(out=t1, in_=t1_d[:, :])
            mt = const.tile([128, 2, 2, P], f8)
            nc.sync.dma_start(
                out=mt,
                in_=mt_d[:, :].rearrange("p (a b c) -> p a b c", a=2, b=2))
            wvt = const.tile([128, 2, 2, P], f8)
            nc.sync.dma_start(
                out=wvt,
                in_=wvt_d[:, :].rearrange("p (a b c) -> p a b c", a=2, b=2))
            pos = const.tile([128, PC, N], bf16)
            nc.sync.dma_start(out=pos, in_=pos_d[:, :, :])
            s2 = const.tile([128, PC], f32)
            nc.sync.dma_start(out=s2, in_=s2_d[:, :])
            t2 = const.tile([128, PC], f32)
            nc.sync.dma_start(out=t2, in_=t2_d[:, :])
            # x(pair1) ahead of w3 so pair1's conv1 isn't DMA-gated
            x1_t = xp.tile([128, KC1, N2], bf16, name="x_1", tag="x")
            nc.sync.dma_start(
                out=x1_t,
                in_=x_d[:, 1, :].rearrange("p (k n) -> p k n", k=KC1))
            w3t = const.tile([128, 2, 2, CIN], f8)
            nc.sync.dma_start(
                out=w3t,
                in_=w3t_d[:, :].rearrange("p (a b c) -> p a b c", a=2, b=2))
            t3 = const.tile([128, KC1], f32)
            nc.sync.dma_start(out=t3, in_=t3_d[:, :])

            ident = const.tile([128, 128], bf16)
            make_identity(nc, ident)
            # 32*I: adds the residual x into conv3's PSUM, pre-scaled to
            # match the w3 fp8 host-scale of 32 (undone in the relu).
            ident32 = const.tile([128, 128], bf16)
            nc.gpsimd.memset(ident32, 0.0)
            nc.gpsimd.affine_select(
                out=ident32, in_=ident32,
                compare_op=mybir.AluOpType.not_equal, fill=32.0, base=0,
                pattern=[[-1, 128]], channel_multiplier=1)

            Exp = mybir.ActivationFunctionType.Exp
            Relu = mybir.ActivationFunctionType.Relu
            Copy = mybir.ActivationFunctionType.Copy
            Add = mybir.AluOpType.add
            Mult = mybir.AluOpType.mult
            Max = mybir.AluOpType.max

            # per-pair x access plans; pair 0/1 tiles were DMA'd above
            xparts_all = {
                0: [(x0c[kc // KCC], kc % KCC) for kc in range(KC1)],
                1: [(x1_t, kc) for kc in range(KC1)],
            }

            def fetch_x(pair):
                if pair in xparts_all:
                    return
                x_t = xp.tile([128, KC1, N2], bf16, name=f"x_{pair}",
                              tag="x")
                nc.sync.dma_start(
                    out=x_t,
                    in_=x_d[:, pair, :].rearrange("p (k n) -> p k n",
                                                  k=KC1))
                xparts_all[pair] = [(x_t, kc) for kc in range(KC1)]

            h1_tiles = {}

            def conv1_oc(pair, oc):
                # one conv1 output-chunk: matmuls + bn1/relu into h1 (bf16)
                # and an fp8 DoubleRow-layout copy for the g/v projections
                h1, h1_8 = h1_tiles[pair]
                xparts = xparts_all[pair]
                cps = ps_mm.tile([128, 512], f32, name="cps", tag="mm")
                for kc in range(KC1):
                    wtile, wi = w1c[kc // KCC], kc % KCC
                    xtile, xi = xparts[kc]
                    nc.tensor.matmul(
                        cps[:, :N2],
                        wtile[:, wi, oc * 128:(oc + 1) * 128],
                        xtile[:, xi, :],
                        start=(kc == 0), stop=(kc == KC1 - 1),
                    )
                nc.scalar.activation(h1[:, oc, :], cps[:, :N2], Relu,
                                     bias=t1[:, oc:oc + 1])
                nc.vector.tensor_copy(h1_8[:, oc // 2, oc % 2, :N2],
                                      h1[:, oc, :])

            def conv1_alloc(pair):
                h1 = h1p.tile([128, PC, N2], bf16, name=f"h1_{pair}",
                              tag="h1")
                h1_8 = h1p.tile([128, 2, 2, N2P], f8, name=f"h18_{pair}",
                                tag="h18")
                nc.vector.memset(h1_8[:, :, :, N2:], 0.0)
                h1_tiles[pair] = (h1, h1_8)

            # pair 0's conv1 runs standalone (gated by the startup DMAs);
            # conv1 of pair p+1 is emitted inside pair p's attention to fill
            # the PE while softmax chains resolve.
            conv1_alloc(0)
            for oc in range(PC):
                conv1_oc(0, oc)

            for pair in range(NPAIR):
                h1, h1_8 = h1_tiles[pair]
                nxt = pair + 1 if pair + 1 < NPAIR else None
                if nxt is not None:
                    fetch_x(nxt)
                    conv1_alloc(nxt)
                filler = list(range(PC)) if nxt is not None else []

                # ---- score projection g = (Wq^T Wk)^T h1 (fp8 DoubleRow)
                g_sb = qkp.tile([128, PC, N2], bf16, name=f"g_{pair}",
                                tag="g")
                for oc in range(PC):
                    qps = ps_mm.tile([128, 512], f32, name="qps", tag="mm")
                    for dp in range(2):
                        nc.tensor.matmul(
                            qps[:, :N2],
                            mt[:, dp, :, oc * 128:(oc + 1) * 128],
                            h1_8[:, dp, :, :N2],
                            start=(dp == 0), stop=(dp == 1),
                            perf_mode=DR,
                        )
                    nc.vector.tensor_scalar_mul(g_sb[:, oc, :],
                                                qps[:, :N2], 1.0 / 64)

                # ---- per-image attention ----
                vT_list = []
                attnT_list = []
                for j in range(2):
                    # v^T directly: vT[m, c] = sum_p h1[p, m] wvt[p, c]
                    vT = attp.tile([128, 2, P], bf16, name=f"vT_{pair}_{j}",
                                   tag="vT")
                    for mi, (m0, msz) in enumerate(NCHUNKS):
                        vps = ps_mm.tile([128, 512], f32, name="vps",
                                         tag="mm")
                        for dp in range(2):
                            nc.tensor.matmul(
                                vps[:msz, :],
                                h1_8[:, dp, :,
                                     j * N + m0:j * N + m0 + msz],
                                wvt[:, dp, :, :],
                                start=(dp == 0), stop=(dp == 1),
                                perf_mode=DR,
                            )
                        nc.vector.tensor_scalar_mul(vT[:msz, mi, :],
                                                    vps[:msz, :], 1.0 / 32)
                    vT_list.append(vT)

                for j in range(2):
                    # attn^T [128, 2, 196] bf16
                    attnT = attp.tile([128, 2, N], bf16,
                                      name=f"aT_{pair}_{j}", tag="attnT")

                    for ni, (n0, nsz) in enumerate(NCHUNKS):
                        lps = ps_sm.tile([128, N], f32, name="lps",
                                         tag="small")
                        # cc: sum_d h1[d, n-slice]^T g[d, :]
                        for pc in range(PC):
                            nc.tensor.matmul(
                                lps[:nsz, :],
                                h1[:, pc, j * N + n0:j * N + n0 + nsz],
                                g_sb[:, pc, j * N:(j + 1) * N],
                                start=(pc == 0), stop=False,
                            )
                        # cp: sum_d P2[d, n-slice]^T h1[d, :]
                        for pc in range(PC):
                            nc.tensor.matmul(
                                lps[:nsz, :],
                                pos[:, pc, n0:n0 + nsz],
                                h1[:, pc, j * N:(j + 1) * N],
                                start=False, stop=(pc == PC - 1),
                            )
                        # softmax over free dim (logits are O(40) max, exp
                        # stays finite in fp32; no max-subtraction needed)
                        p_raw = attp.tile([128, N], f32, name="p_raw",
                                          tag="p_raw")
                        ssum = attp.tile([128, 1], f32, name="ssum",
                                         tag="ss")
                        nc.scalar.activation(p_raw[:nsz, :], lps[:nsz, :],
                                             Exp, accum_out=ssum[:nsz, :])
                        rsum = attp.tile([128, 1], f32, name="rsum",
                                         tag="rs")
                        nc.vector.reciprocal(rsum[:nsz, :], ssum[:nsz, :])
                        p_nrm = attp.tile([128, N], bf16, name="p_nrm",
                                          tag="p_nrm")
                        nc.vector.tensor_scalar_mul(p_nrm[:nsz, :],
                                                    p_raw[:nsz, :],
                                                    rsum[:nsz, :])
                        # fill the PE with a conv1 chunk of the next pair
                        # while the softmax chain resolves
                        if filler:
                            conv1_oc(nxt, filler.pop(0))
                        elif nxt is None:
                            # last pair: no real work left — issue dummy
                            # matmuls so the PE stays busy and HAM does not
                            # re-throttle the clock before conv3
                            dps = ps_tr.tile([128, 128], bf16, name="dmy",
                                             tag="tr")
                            for _ in range(8):
                                nc.tensor.transpose(dps[:, :],
                                                    ident[:, :],
                                                    ident[:, :])
                        # transpose normalized attn into attnT[m, n-slice]
                        for mi, (m0, msz) in enumerate(NCHUNKS):
                            tps = ps_tr.tile([128, 128], bf16, name="tps",
                                             tag="tr")
                            nc.tensor.transpose(tps[:msz, :nsz],
                                                p_nrm[:nsz, m0:m0 + msz],
                                                ident[:nsz, :nsz])
                            nc.vector.tensor_copy(
                                attnT[:msz, mi, n0:n0 + nsz],
                                tps[:msz, :nsz])
                    attnT_list.append(attnT)

                # ---- attention output + bn2 + relu -> h2 (fp8, DoubleRow
                # layout [dp, i, j*n] padded to 400) ----
                h2 = h2p.tile([128, 2, 2, N2P], f8, name=f"h2_{pair}",
                              tag="h2")
                nc.vector.memset(h2[:, :, :, N2:], 0.0)
                for j in range(2):
                    vT = vT_list[j]
                    attnT = attnT_list[j]
                    for c4 in range(PC):
                        aps = ps_sm.tile([128, N], f32, name="aps",
                                         tag="small")
                        for mi, (m0, msz) in enumerate(NCHUNKS):
                            nc.tensor.matmul(
                                aps[:, :],
                                vT[:msz, mi, c4 * 128:(c4 + 1) * 128],
                                attnT[:msz, mi, :],
                                start=(mi == 0), stop=(mi == 1),
                            )
                        nc.scalar.activation(
                            h2[:, c4 // 2, c4 % 2, j * N:(j + 1) * N],
                            aps[:, :], Relu, bias=t2[:, c4:c4 + 1],
                            scale=s2[:, c4:c4 + 1])

                # ---- conv3 (fp8 DoubleRow) + bn3 + residual + relu -> y ----
                ysb = outp.tile([128, KC1, N2], bf16, name=f"y_{pair}",
                                tag="ysb")
                for oc in range(KC1):
                    ops = ps_mm.tile([128, 512], f32, name="ops", tag="mm")
                    for dp in range(2):
                        nc.tensor.matmul(
                            ops[:, :N2],
                            w3t[:, dp, :, oc * 128:(oc + 1) * 128],
                            h2[:, dp, :, :N2],
                            start=(dp == 0), stop=False,
                            perf_mode=DR,
                        )
                    xtile, xi = xparts_all[pair][oc]
                    # accumulate the residual 32*x into PSUM on the PE, so
                    # the epilogue is a single activation (no vector op)
                    nc.tensor.matmul(
                        ops[:, :N2], ident32, xtile[:, xi, :],
                        start=False, stop=True, skip_group_check=True,
                    )
                    nc.scalar.activation(ysb[:, oc, :], ops[:, :N2], Relu,
                                         scale=1.0 / 32,
                                         bias=t3[:, oc:oc + 1])
                    if oc % 2 == 1:
                        g = oc - 1
                        nc.sync.dma_start(
                            out=y_d[:, pair, g * N2:(g + 2) * N2].rearrange(
                                "p (k n) -> p k n", k=2),
                            in_=ysb[:, g:g + 2, :])

    nc.compile()
    return nc


def _prep_inputs(x, w1, g1, b1, m1, v1, wqkv, rel_h, rel_w,
                 g2, b2, m2, v2, w3, g3, b3, m3, v3):
    f = np.float32
    s1 = (g1 / np.sqrt(v1 + EPS)).astype(f)
    t1 = (b1 - m1 * s1).astype(f)
    s2 = (g2 / np.sqrt(v2 + EPS)).astype(f)
    t2 = (b2 - m2 * s2).astype(f)
    s3 = (g3 / np.sqrt(v3 + EPS)).astype(f)
    t3 = (b3 - m3 * s3).astype(f)

    w1p = (w1 * s1[:, None]).astype(f)                    # [512, 2048]
    w1t = np.ascontiguousarray(
        w1p.T.reshape(KC1, 128, P).transpose(1, 0, 2)).astype(BF16)
    # Fold q/k projections: cc = h1^T (Wq^T Wk) h1 -> g = MT^T h1 with
    # MT[e, d] = (Wk^T Wq)[e, d]; cp = pos^T q -> P2^T h1, P2 = Wq^T pos.
    wq = wqkv[:P].astype(np.float64)                      # [512, 512]
    wk = wqkv[P:2 * P].astype(np.float64)
    # fp8 weights are host-scaled by powers of 2 out of the subnormal
    # range; the inverse scale is folded into on-chip copies.
    # DoubleRow layout: contraction row = dp*256 + i*128 + p.
    mtm = (wk.T @ wq) * 64                                # [512e, 512d]
    mt = np.ascontiguousarray(
        mtm.reshape(2, 2, 128, P).transpose(2, 0, 1, 3)).reshape(
        128, 2 * 2 * P).astype(F8E4)
    wv = wqkv[2 * P:].astype(np.float64) * 32             # [512, 512]
    wvt = np.ascontiguousarray(
        wv.T.reshape(2, 2, 128, P).transpose(2, 0, 1, 3)).reshape(
        128, 2 * 2 * P).astype(F8E4)
    w3p = (w3 * s3[:, None]).astype(f) * 32               # [2048, 512]
    w3t = np.ascontiguousarray(
        w3p.T.reshape(2, 2, 128, CIN).transpose(2, 0, 1, 3)).reshape(
        128, 2 * 2 * CIN).astype(F8E4)
    posm = (np.asarray(rel_h, np.float64)
            + np.asarray(rel_w, np.float64)).reshape(P, N)
    p2 = np.asarray(wqkv[:P], np.float64).T @ posm        # [512d, 196n]
    pos = np.ascontiguousarray(
        p2.reshape(PC, 128, N).transpose(1, 0, 2)).astype(BF16)

    t1_h = np.ascontiguousarray(t1.reshape(PC, 128).T)
    s2_h = np.ascontiguousarray(s2.reshape(PC, 128).T)
    t2_h = np.ascontiguousarray(t2.reshape(PC, 128).T)
    t3_h = np.ascontiguousarray(t3.reshape(KC1, 128).T)

    shared = dict(w1t=w1t, mt=mt, wvt=wvt, w3t=w3t, pos=pos,
                  t1=t1_h, s2=s2_h, t2=t2_h, t3=t3_h)

    xb = np.asarray(x, f).astype(BF16)
    in_maps = []
    for c in range(NCORES):
        # [BPC, CIN, H, W] -> [128, NPAIR, KC1 * N2] with free layout
        # per pair: [kc, j, n]
        xc = xb[c * BPC:(c + 1) * BPC].reshape(NPAIR, 2, KC1, 128, N)
        xc = np.ascontiguousarray(xc.transpose(3, 0, 2, 1, 4)).reshape(
            128, NPAIR, KC1 * N2)
        in_maps.append(dict(shared, x=xc))
    return in_maps


def _run(in_maps, trace=False):
    from concourse.bass_utils import run_bass_kernel_spmd
    if "nc" not in _CACHE:
        _CACHE["nc"] = _build()
    nc = _CACHE["nc"]
    return run_bass_kernel_spmd(nc, in_maps, core_ids=list(range(NCORES)),
                                trace=trace)


def _assemble(results):
    out = np.empty((B, CIN, H, W), np.float32)
    for c in range(NCORES):
        # [128, NPAIR, KC1*N2] -> [BPC, CIN, H, W]
        yc = results[c]["y"].reshape(128, NPAIR, KC1, 2, N).astype(
            np.float32)
        out[c * BPC:(c + 1) * BPC] = yc.transpose(1, 3, 2, 0, 4).reshape(
            BPC, CIN, H, W)
    return out


def kernel(**inputs):
    in_maps = _prep_inputs(**inputs)
    res = _run(in_maps)
    return _assemble(res.results)
